# revision 11
# baseline (speedup 1.0000x reference)
"""Trainium2 Bass kernel for pre-LN multi-head attention with null-KV (v2).

Same math and sharding as the baseline kernel (8 cores = 4 batches x 2
head-groups; host sums the two partial y's per batch), restructured so the
Activation engine (exp = 33.5M elements/core, the true bottleneck) is busy
from ~25us onward instead of idling through a 178us projection pre-phase.

Key changes vs baseline:
- Staged pipeline: token quarter Q's LN/transpose/K,V,Q-projections are
  interleaved into attention stage Q-1. Attention unit (c, g) = q-chunk c
  vs key-group g (4 key tiles), scheduled at stage max(c, g).
- attn@v partial sums accumulate in PSUM within a stage-run and spill to
  fp16 SBUF accumulators across stages (chunk 3 never spills: its 4 units
  are consecutive at stage 3 and normalize straight from PSUM).
- All matmuls fp16 (1 cycle/row, FWL-eligible); transposes fp16 (1 c/row,
  was fp32 at 2 c/row). LayerNorm rstd via DVE bit-trick rsqrt + 2 Newton
  steps so the Act engine runs exp only (single act-table set, no thrash).
- PSUM: 8 banks = "simy" tag 3 x [128,1024]f32 (sims; also lends slots to
  proj/transpose/yproj/null-logit tiles) + "acc" tag 2 x [128,512]f32.
"""

import sys

sys.path.insert(0, "/opt/trn_rl_repo")

import numpy as np

HEADS = 16
DIM_HEAD = 64
DIM = 1024
INNER = HEADS * DIM_HEAD
SCALE = DIM_HEAD ** -0.5

N_TOK = 2048
HC = 512          # head-cols per core (8 heads x 64)
NHEAD = 8
NPAIR = 4
NKT = 16          # key tiles of 128
NQC = 4           # query chunks of 512
KC = 8            # contraction chunks of 128 over DIM
NG = 4            # key groups (4 kt each) == stages

_CACHE: dict = {}

RSQRT_MAGIC = 0x5F3759DF


def _build_nc():
    from contextlib import ExitStack

    import concourse.bacc as bacc
    import concourse.bass as bass
    import concourse.tile as tile
    from concourse import mybir

    f32 = mybir.dt.float32
    f16 = mybir.dt.float16
    i32 = mybir.dt.int32
    AF = mybir.ActivationFunctionType
    ALU = mybir.AluOpType
    PSUM = bass.MemorySpace.PSUM

    nc = bacc.Bacc(None)

    x_d = nc.declare_dram_parameter("x", [N_TOK, DIM], f32, isOutput=False)
    wq_d = nc.declare_dram_parameter("wq", [DIM, HC], f16, isOutput=False)
    wk_d = nc.declare_dram_parameter("wk", [DIM, HC], f16, isOutput=False)
    wv_d = nc.declare_dram_parameter("wv", [DIM, HC], f16, isOutput=False)
    wo_d = nc.declare_dram_parameter("wo", [HC, DIM], f16, isOutput=False)
    vones_d = nc.declare_dram_parameter("vones", [128, NHEAD], f16, isOutput=False)
    nullk_d = nc.declare_dram_parameter("nullk", [128, NHEAD], f16, isOutput=False)
    nullv_d = nc.declare_dram_parameter("nullv", [1, NHEAD, 65], f16, isOutput=False)
    ident_d = nc.declare_dram_parameter("ident", [128, 128], f16, isOutput=False)
    rsqc_d = nc.declare_dram_parameter("rsqc", [128, 2], i32, isOutput=False)
    y_d = nc.declare_dram_parameter("y", [N_TOK, DIM], f32, isOutput=True)

    with tile.TileContext(nc) as tc, ExitStack() as ctx:
        pers = ctx.enter_context(tc.tile_pool(name="pers", bufs=1))
        ps = ctx.enter_context(tc.tile_pool(name="ps", bufs=3, space=PSUM))
        xpool = ctx.enter_context(tc.tile_pool(name="xpool", bufs=4))
        xnpool = ctx.enter_context(tc.tile_pool(name="xnpool", bufs=4))
        xnt = ctx.enter_context(tc.tile_pool(name="xnt", bufs=1))
        stat = ctx.enter_context(tc.tile_pool(name="stat", bufs=4))
        ppool = ctx.enter_context(tc.tile_pool(name="ppool", bufs=5))
        pnpool = ctx.enter_context(tc.tile_pool(name="pnpool", bufs=8))
        rcpool = ctx.enter_context(tc.tile_pool(name="rcpool", bufs=2))
        rbpool = ctx.enter_context(tc.tile_pool(name="rbpool", bufs=2))
        smpool = ctx.enter_context(tc.tile_pool(name="smpool", bufs=1))
        ypool = ctx.enter_context(tc.tile_pool(name="ypool", bufs=3))

        # ---------------- persistent tiles + initial DMAs -------------------
        ident_sb = pers.tile([128, 128], f16, tag="ident", name="ident")
        nc.sync.dma_start(out=ident_sb, in_=ident_d[:, :])
        rsqc_sb = pers.tile([128, 2], i32, tag="rsqc", name="rsqc")
        nc.sync.dma_start(out=rsqc_sb, in_=rsqc_d[:, :])
        nullk_sb = pers.tile([128, NHEAD], f16, tag="nullk", name="nullk")
        nc.sync.dma_start(out=nullk_sb, in_=nullk_d[:, :])
        nullv_sb = pers.tile([1, NHEAD, 65], f16, tag="nullv", name="nullv")
        nc.sync.dma_start(out=nullv_sb, in_=nullv_d[:, :, :])

        # all four pairs' qT in one tile so the batched null-logit matmul can
        # stream two pairs per rhs AP
        qT_all = pers.tile([128, NPAIR, N_TOK], f16, tag="qT", name="qT")

        class _QTView:
            def __getitem__(self, pr):
                return qT_all[:, pr, :]

        qT = _QTView()
        kT = [pers.tile([128, N_TOK], f16, tag=f"kT{i}", name=f"kT{i}")
              for i in range(NPAIR)]
        # head h's v in cols h*65..h*65+64, fused ones col at h*65+64; 584
        # wide so the 128-col lhsT slice stays in bounds for every head
        v_sb = [pers.tile([128, 584], f16, tag=f"v{i}", name=f"v{i}")
                for i in range(NKT)]
        outT = [pers.tile([128, N_TOK], f16, tag=f"outT{i}", name=f"outT{i}")
                for i in range(NPAIR)]
        # cross-stage attn@v accumulators for chunks 0-2 (chunk 3 stays in
        # PSUM): h0 -> cols 0:512, h1 -> cols 512:1024, rows 0-63 = v-dims,
        # row 64 = softmax denominator
        accS = {(c, pr): pers.tile([128, 1024], f16, tag=f"aS{c}{pr}",
                                   name=f"aS{c}{pr}")
                for c in range(3) for pr in range(NPAIR)}

        # first quarter's x queued before the weights so LN starts at once
        x_tiles: dict = {}
        for t4 in range(4):
            x_t = xpool.tile([128, DIM], f32, tag="x", name="x")
            nc.sync.dma_start(out=x_t, in_=x_d[t4 * 128:(t4 + 1) * 128, :])
            x_tiles[t4] = x_t

        wq_sb = [pers.tile([128, HC], f16, tag=f"wq{k}", name=f"wq{k}") for k in range(KC)]
        wk_sb = [pers.tile([128, HC], f16, tag=f"wk{k}", name=f"wk{k}") for k in range(KC)]
        wv_sb = [pers.tile([128, HC], f16, tag=f"wv{k}", name=f"wv{k}") for k in range(KC)]
        for k in range(KC):
            sl = slice(k * 128, (k + 1) * 128)
            nc.sync.dma_start(out=wk_sb[k], in_=wk_d[sl, :])
            nc.sync.dma_start(out=wv_sb[k], in_=wv_d[sl, :])
        for k in range(KC):
            sl = slice(k * 128, (k + 1) * 128)
            nc.sync.dma_start(out=wq_sb[k], in_=wq_d[sl, :])
        wo_sb = [pers.tile([128, DIM], f16, tag=f"wo{i}", name=f"wo{i}")
                 for i in range(NPAIR)]
        for i in range(NPAIR):
            nc.sync.dma_start(out=wo_sb[i], in_=wo_d[i * 128:(i + 1) * 128, :])
        for t in range(NKT):
            nc.sync.dma_start(
                out=v_sb[t][:, 0:520].rearrange("p (h e) -> p h e", e=65)[:, :, 64:65],
                in_=vones_d[:, :])

        # ---------------- emission helpers ----------------------------------
        xnT_all: dict = {}       # Q -> [128, KC, 512] f16
        pn_tiles: dict = {}      # (c, pr) -> [1, 2, 512] f16
        run_acc: dict = {}       # (c, pr) -> [acc_h0, acc_h1] PSUM or None
        run_started: dict = {}   # (c, pr) -> bool (acc group has start inst)
        pend: list = []          # LAG queue of pending attn@v steps
        normed = {c: 0 for c in range(NQC)}
        task_q: list = []        # deferred PE-side task closures

        def emit_stats(Q, t4):
            """DVE/DMA-only part of LN for x tile Q*4+t4 (emit early)."""
            tt = Q * 4 + t4
            if tt not in x_tiles:
                x_t = xpool.tile([128, DIM], f32, tag="x", name="x")
                nc.sync.dma_start(out=x_t, in_=x_d[tt * 128:(tt + 1) * 128, :])
                x_tiles[tt] = x_t
            x_t = x_tiles[tt]
            st6 = stat.tile([128, 2, 6], f32, tag="st6", name="st6")
            nc.vector.bn_stats(out=st6[:, 0, :], in_=x_t[:, 0:512])
            nc.vector.bn_stats(out=st6[:, 1, :], in_=x_t[:, 512:1024])
            mv = stat.tile([128, 2], f32, tag="mv", name="mv")
            nc.vector.bn_aggr(out=mv, in_=st6)
            # rstd = rsqrt(var + 1e-5) on DVE: magic-constant seed + 2 Newton
            ve = stat.tile([128, 1], f32, tag="ve", name="ve")
            nc.vector.tensor_scalar(out=ve, in0=mv[:, 1:2], scalar1=1e-5,
                                    scalar2=None, op0=ALU.add)
            iy = stat.tile([128, 1], i32, tag="iy", name="iy")
            nc.vector.tensor_scalar(out=iy, in0=ve.bitcast(i32),
                                    scalar1=rsqc_sb[:, 1:2], scalar2=None,
                                    op0=ALU.arith_shift_right)
            nc.vector.tensor_tensor(out=iy, in0=rsqc_sb[:, 0:1], in1=iy,
                                    op=ALU.subtract)
            y0 = iy.bitcast(f32)
            t1 = stat.tile([128, 1], f32, tag="t1", name="t1")
            rstd = stat.tile([128, 1], f32, tag="rstd", name="rstd")
            cur = y0
            for it in range(2):
                nc.vector.tensor_tensor(out=t1, in0=cur, in1=cur, op=ALU.mult)
                nc.vector.tensor_tensor(out=t1, in0=t1, in1=ve, op=ALU.mult)
                nc.vector.tensor_scalar(out=t1, in0=t1, scalar1=-0.5,
                                        scalar2=1.5, op0=ALU.mult,
                                        op1=ALU.add)
                nc.vector.tensor_tensor(out=rstd, in0=cur, in1=t1, op=ALU.mult)
                cur = rstd
            xn_t = xnpool.tile([128, DIM], f16, tag="xn", name="xn")
            nc.vector.tensor_scalar(out=xn_t, in0=x_t, scalar1=mv[:, 0:1],
                                    scalar2=rstd, op0=ALU.subtract,
                                    op1=ALU.mult)
            del x_tiles[tt]
            return xn_t

        def emit_tp(Q, t4, xn_t):
            """Transpose one normalized x tile into xnT_all[Q] via regular
            matmuls against an identity rhs (N=128 issue rate ~81ns vs
            ~275ns for PE transpose-mode, and it engages HAM warm)."""
            if Q not in xnT_all:
                xnT_all[Q] = xnt.tile([128, KC, 512], f16, tag="xnt",
                                      name=f"xnT{Q}")
            tp = ps.tile([128, 1024], f32, tag="simy", name="tp")
            for fc in range(KC):
                nc.tensor.matmul(tp[:, fc * 128:(fc + 1) * 128],
                                 lhsT=xn_t[:, fc * 128:(fc + 1) * 128],
                                 rhs=ident_sb, start=True, stop=True)
            nc.vector.tensor_copy(
                out=xnT_all[Q][:, :, t4 * 128:(t4 + 1) * 128],
                in_=tp.rearrange("p (k t) -> p k t", t=128))

        def emit_proj(Q, w_sb, dst, ct):
            """q/k projection chunk: dst[ct][:, Q-cols] = (w.T @ xn.T)."""
            pp = ps.tile([128, HC], f32, tag="simy", name="pp")
            for k in range(KC):
                nc.tensor.matmul(pp, lhsT=w_sb[k][:, ct * 128:(ct + 1) * 128],
                                 rhs=xnT_all[Q][:, k, :],
                                 start=(k == 0), stop=(k == KC - 1))
            nc.vector.tensor_copy(out=dst[ct][:, Q * 512:(Q + 1) * 512], in_=pp)

        def emit_vproj(Q, t4):
            tt = Q * 4 + t4
            pp = ps.tile([128, HC], f32, tag="simy", name="ppv")
            for k in range(KC):
                nc.tensor.matmul(pp,
                                 lhsT=xnT_all[Q][:, k, t4 * 128:(t4 + 1) * 128],
                                 rhs=wv_sb[k],
                                 start=(k == 0), stop=(k == KC - 1))
            nc.vector.tensor_copy(
                out=v_sb[tt][:, 0:520].rearrange("p (h e) -> p h e", e=65)[:, :, 0:64],
                in_=pp.rearrange("p (h d) -> p h d", h=NHEAD))

        def emit_pn(c, pr):
            """null-key logits + exp for chunk c, pair pr."""
            pnt = pnpool.tile([1, 2, 512], f16, tag="pn", name="pn")
            pn_tiles[(c, pr)] = pnt
            for h2 in range(2):
                h = pr * 2 + h2
                ro = h2 * 64
                pnp = ps.tile([128, 512], f32, tag="simy", name="pnp")
                nc.tensor.matmul(
                    pnp[0:1, :], lhsT=nullk_sb[ro:ro + 64, h:h + 1],
                    rhs=qT_all[ro:ro + 64, pr, c * 512:(c + 1) * 512],
                    start=True, stop=True)
                nc.scalar.activation(out=pnt[0:1, h2, :], in_=pnp[0:1, :],
                                     func=AF.Exp)

        def emit_sims(c, g, pr, u):
            kt0 = g * 4 + u * 2
            ccols = slice(c * 512, (c + 1) * 512)
            sims = [ps.tile([128, 1024], f32, tag="simy", name="sim")
                    for _ in range(2)]
            for j in range(2):
                kt = kt0 + j
                for h2 in range(2):
                    ro = h2 * 64
                    nc.tensor.matmul(
                        sims[h2][:, j * 512:(j + 1) * 512],
                        lhsT=kT[pr][ro:ro + 64, kt * 128:(kt + 1) * 128],
                        rhs=qT[pr][ro:ro + 64, ccols], start=True, stop=True)
            Ps = []
            for h2 in range(2):
                p_sb = ppool.tile([128, 1024], f16, tag="P", name="P")
                nc.scalar.activation(out=p_sb, in_=sims[h2], func=AF.Exp)
                Ps.append(p_sb)
            pend.append((c, g, pr, u, kt0, Ps))

        def run_end_g(c, g):
            """Last g of the PSUM run containing g for chunk c."""
            return c if g <= c else g

        def emit_drain_or_norm(c, g, pr):
            ge = run_end_g(c, g)
            accs = run_acc[(c, pr)]
            if ge == 3:
                emit_norm(c, pr, accs)
            else:
                first = ge == c
                for h2 in range(2):
                    cols = slice(h2 * 512, (h2 + 1) * 512)
                    if first:
                        nc.vector.tensor_copy(out=accS[(c, pr)][0:65, cols],
                                              in_=accs[h2][0:65, :])
                    else:
                        nc.vector.tensor_tensor(out=accS[(c, pr)][0:65, cols],
                                                in0=accs[h2][0:65, :],
                                                in1=accS[(c, pr)][0:65, cols],
                                                op=ALU.add)
            run_acc[(c, pr)] = None
            run_started[(c, pr)] = False

        def emit_norm(c, pr, accs):
            ccols = slice(c * 512, (c + 1) * 512)
            for h2 in range(2):
                ro = h2 * 64
                cols = slice(h2 * 512, (h2 + 1) * 512)
                zf = rcpool.tile([1, 512], f32, tag="zf", name="zf")
                if c < 3:
                    nc.vector.tensor_tensor(out=zf, in0=accs[h2][64:65, :],
                                            in1=accS[(c, pr)][64:65, cols],
                                            op=ALU.add)
                else:
                    nc.vector.tensor_copy(out=zf, in_=accs[h2][64:65, :])
                rc = rcpool.tile([1, 512], f32, tag="rc", name="rc")
                nc.vector.reciprocal_approx_fast(out=rc, in_=zf)
                rb = rbpool.tile([64, 512], f32, tag="rb", name="rb")
                nc.gpsimd.partition_broadcast(rb, rc, channels=64)
                if c < 3:
                    sm = smpool.tile([64, 512], f32, tag="sm", name="sm")
                    nc.vector.tensor_tensor(out=sm, in0=accs[h2][0:64, :],
                                            in1=accS[(c, pr)][0:64, cols],
                                            op=ALU.add)
                    nc.vector.tensor_tensor(out=outT[pr][ro:ro + 64, ccols],
                                            in0=sm, in1=rb, op=ALU.mult)
                else:
                    nc.vector.tensor_tensor(out=outT[pr][ro:ro + 64, ccols],
                                            in0=accs[h2][0:64, :], in1=rb,
                                            op=ALU.mult)
            normed[c] += 1
            if normed[c] == NPAIR:
                for t4 in range(4):
                    for nh in range(2):
                        task_q.append(_yproj_task(c, t4, nh))

        def _yproj_task(c, t4, nh):
            def task():
                tt = c * 4 + t4
                yp = ps.tile([128, 512], f32, tag="simy", name="yp")
                for ic in range(NPAIR):
                    nc.tensor.matmul(
                        yp, lhsT=outT[ic][:, tt * 128:(tt + 1) * 128],
                        rhs=wo_sb[ic][:, nh * 512:(nh + 1) * 512],
                        start=(ic == 0), stop=(ic == NPAIR - 1))
                y_sb = ypool.tile([128, 512], f32, tag="ysb", name="ysb")
                nc.vector.tensor_copy(out=y_sb, in_=yp)
                nc.sync.dma_start(
                    out=y_d[tt * 128:(tt + 1) * 128,
                            nh * 512:(nh + 1) * 512],
                    in_=y_sb)
            return task

        def emit_attnv_step():
            if not pend:
                return
            c, g, pr, u, kt0, Ps = pend.pop(0)
            if run_acc.get((c, pr)) is None:
                accs = [ps.tile([128, 512], f32, tag="acc", name="acc",
                                bufs=2) for _ in range(2)]
                run_acc[(c, pr)] = accs
                run_started[(c, pr)] = False
                if g == 0:
                    for h2 in range(2):
                        h = pr * 2 + h2
                        nc.tensor.matmul(accs[h2][0:65, :],
                                         lhsT=nullv_sb[0:1, h, :],
                                         rhs=pn_tiles[(c, pr)][0:1, h2, :],
                                         start=True, stop=False)
                    run_started[(c, pr)] = True
            accs = run_acc[(c, pr)]
            ge = run_end_g(c, g)
            last_u = g == ge and u == 1
            for h2 in range(2):
                h = pr * 2 + h2
                for j in range(2):
                    kt = kt0 + j
                    st = not run_started[(c, pr)] and j == 0
                    nc.tensor.matmul(accs[h2],
                                     lhsT=v_sb[kt][:, h * 65:h * 65 + 128],
                                     rhs=Ps[h2][:, j * 512:(j + 1) * 512],
                                     start=st, stop=(last_u and j == 1))
            run_started[(c, pr)] = True
            if last_u:
                emit_drain_or_norm(c, g, pr)

        def emit_subunit(c, g, pr):
            # one u-step of LAG: attn@v trails sims so the PE never
            # head-of-line blocks on the exp of the sims it just issued
            for u in range(2):
                emit_sims(c, g, pr, u)
                if len(pend) > 1:
                    emit_attnv_step()

        # ---------------- schedule -------------------------------------------
        # prephase for quarter 0 (before any attention)
        xns = [emit_stats(0, t4) for t4 in range(4)]
        for t4 in range(4):
            emit_tp(0, t4, xns[t4])
        for ct in range(NPAIR):
            emit_proj(0, wk_sb, kT, ct)
        for t4 in range(4):
            emit_vproj(0, t4)
        for ct in range(NPAIR):
            emit_proj(0, wq_sb, qT, ct)
        for pr in range(NPAIR):
            emit_pn(0, pr)

        for s in range(4):
            # subunit order: continuation units of old chunks first, then
            # chunk s's contiguous PSUM run (pr-major)
            subs = [(c, s, pr) for c in range(s) for pr in range(NPAIR)]
            subs += [(s, g, pr) for pr in range(NPAIR) for g in range(s + 1)]
            if s < 3:
                Q = s + 1
                xns = [emit_stats(Q, t4) for t4 in range(4)]
                for t4 in range(4):
                    task_q.append((lambda Q=Q, t4=t4, xn=xns[t4]:
                                   emit_tp(Q, t4, xn)))
                for ct in range(NPAIR):
                    task_q.append(lambda Q=Q, ct=ct:
                                  emit_proj(Q, wk_sb, kT, ct))
                for t4 in range(4):
                    task_q.append(lambda Q=Q, t4=t4: emit_vproj(Q, t4))
                for ct in range(NPAIR):
                    task_q.append(lambda Q=Q, ct=ct:
                                  emit_proj(Q, wq_sb, qT, ct))
                for pr in range(NPAIR):
                    task_q.append(lambda Q=Q, pr=pr: emit_pn(Q, pr))
            # interleave: spread queued tasks across this stage's remaining
            # subunits (recomputed each step — yproj tasks arrive mid-stage)
            for i, sub in enumerate(subs):
                emit_subunit(*sub)
                n_rem = len(subs) - i - 1
                k = -(-len(task_q) // (n_rem + 1))
                for _ in range(min(k, len(task_q))):
                    task_q.pop(0)()
            if s == 3:
                while pend:
                    emit_attnv_step()
                while task_q:
                    task_q.pop(0)()

    nc.compile()
    return nc


def _get_nc():
    if "nc" not in _CACHE:
        _CACHE["nc"] = _build_nc()
    return _CACHE["nc"]


def _prep_in_maps(x, gamma, w_q, w_kv, w_out, null_kv):
    x = np.asarray(x, dtype=np.float32)
    gamma = np.asarray(gamma, dtype=np.float32)
    w_q = np.asarray(w_q, dtype=np.float32)
    w_kv = np.asarray(w_kv, dtype=np.float32)
    w_out = np.asarray(w_out, dtype=np.float32)
    null_kv = np.asarray(null_kv, dtype=np.float32)

    g = gamma[:, None]
    wq_full = (g * w_q * SCALE).astype(np.float16)
    wk_full = (g * w_kv[:, :INNER]).astype(np.float16)
    wv_full = (g * w_kv[:, INNER:]).astype(np.float16)
    wo_full = w_out.astype(np.float16)
    ident = np.eye(128, dtype=np.float16)
    rsqc = np.empty((128, 2), dtype=np.int32)
    rsqc[:, 0] = RSQRT_MAGIC
    rsqc[:, 1] = 1

    in_maps = []
    for core in range(8):
        b, gr = core // 2, core % 2
        hs = slice(gr * HC, (gr + 1) * HC)
        nullk = np.zeros((128, NHEAD), dtype=np.float16)
        nullv = np.zeros((1, NHEAD, 65), dtype=np.float16)
        for j in range(NHEAD):
            h = gr * NHEAD + j
            ro = (j % 2) * 64
            nullk[ro:ro + 64, j] = null_kv[0, h, 0, :]
            nullv[0, j, :64] = null_kv[1, h, 0, :]
            nullv[0, j, 64] = 1.0
        in_maps.append({
            "x": np.ascontiguousarray(x[b]),
            "vones": np.ones((128, NHEAD), dtype=np.float16),
            "wq": np.ascontiguousarray(wq_full[:, hs]),
            "wk": np.ascontiguousarray(wk_full[:, hs]),
            "wv": np.ascontiguousarray(wv_full[:, hs]),
            "wo": np.ascontiguousarray(wo_full[hs, :]),
            "nullk": nullk,
            "nullv": nullv,
            "ident": ident,
            "rsqc": rsqc,
        })
    return in_maps


def kernel(x, gamma, w_q, w_kv, w_out, null_kv, _want_results=False):
    from concourse.bass_utils import run_bass_kernel_spmd

    nc = _get_nc()
    in_maps = _prep_in_maps(x, gamma, w_q, w_kv, w_out, null_kv)
    res = run_bass_kernel_spmd(nc, in_maps, list(range(8)))
    outs = [res.results[c]["y"] for c in range(8)]
    y = np.empty((4, N_TOK, DIM), dtype=np.float32)
    for b in range(4):
        np.add(outs[2 * b], outs[2 * b + 1], out=y[b])
    if _want_results:
        return y, res
    return y

